# revision 12
# baseline (speedup 1.0000x reference)
"""BertSelfAttention on 8 Trainium2 NeuronCores.

Problem: B=2, S=2048, H=1024, 16 heads x 64. Sharding: batch x head-group
(2 batches x 4 head-groups of 4 heads = 8 cores). Each core computes
q/k/v projections for its 4 heads and full attention over them.

Single interleaved instruction stream, built around two facts:
 - ACT (exp over all S^2*nhl scores) is the serial floor (~147us);
   it must never wait.
 - The PE drops from 2.4GHz to 1.2GHz whenever it idles >~100ns and
   needs ~3us of continuous work to get back to full clock, so it must
   never idle: projection chains are spread through the attention
   stream as filler work.

Structure: 8 "units" of (head-pair hp, 512 queries). Per unit, 16 key
chunks: scores[128k, 2x512q] (both heads merged; the mask bias is
per-partition = per-key, so one exp instruction covers both heads),
exp -> probs fp16, PV accumulate [65, 512] per head (65th V column = 1
-> softmax denominator). The first two units defer their PV (probs
parked in SBUF) because V isn't projected yet; four later units host
the deferred PV chains. Tails (1/64-scaled fp16 transpose via PE,
reciprocal, normalize, store) are spread into the next unit's stream.

PSUM budget (8 banks): scores ring 2x[128,1024] = 4, pv A/B/D = 3,
proj/transpose scratch = 1.
"""

import sys

sys.path.insert(0, "/opt/trn_rl_repo")

import numpy as np

import concourse.tile as tile
from concourse.masks import make_identity
from concourse import bacc, mybir
from concourse.bass_utils import run_bass_kernel_spmd

F32 = mybir.dt.float32
F16 = mybir.dt.float16
EXP = mybir.ActivationFunctionType.Exp

B, S, H = 2, 2048, 1024
NH, HD = 16, 64
G = 4                 # head-groups (cores per batch)
NHL = NH // G         # heads per core
O = NHL * HD          # 256 output features per core
IC = H // 128         # 8 contraction chunks
KC = S // 128         # 16 key chunks
QC = 512              # queries per unit
NU = S // QC          # 4 q-units per head pair
NJ = QC // 128        # 128-row blocks per unit
NEG = -1.0e30


def build_nc():
    nc = bacc.Bacc(None, target_bir_lowering=False)
    xT = nc.declare_dram_parameter("xT", [H, S], F16, isOutput=False)
    # weights host-packed p-major ([p, i*O]) so DMA lines are 4KB
    wqT = nc.declare_dram_parameter("wqT", [128, IC * O], F16, isOutput=False)
    wkT = nc.declare_dram_parameter("wkT", [128, IC * O], F16, isOutput=False)
    wvT = nc.declare_dram_parameter("wvT", [128, IC * O], F16, isOutput=False)
    # msk: cols 0:16 = additive mask bias per key chunk, 16:20 = q/k biases
    msk = nc.declare_dram_parameter("msk", [128, KC + 4], F32, isOutput=False)
    bvb = nc.declare_dram_parameter("bvb", [128, NHL * (HD + 1)], F16,
                                    isOutput=False)
    out = nc.declare_dram_parameter("out", [S, O], F32, isOutput=True)

    xTr = xT.rearrange("(i p) s -> p i s", p=128)
    wqr = wqT.rearrange("p (i o) -> p i o", i=IC)
    wkr = wkT.rearrange("p (i o) -> p i o", i=IC)
    wvr = wvT.rearrange("p (i o) -> p i o", i=IC)

    with tile.TileContext(nc) as tc:
        with tc.tile_pool(name="consts", bufs=1) as consts, \
             tc.tile_pool(name="persist", bufs=1) as persist, \
             tc.tile_pool(name="pbdp", bufs=1) as pbdp:
            ident = consts.tile([128, 128], F16, tag="ident")
            make_identity(nc, ident)
            msk_sb = consts.tile([128, KC + 4], F32, tag="msk")
            bvb_sb = consts.tile([128, NHL * (HD + 1)], F16, tag="bvb")

            xt = persist.tile([128, IC, S], F16, tag="xt")
            wq = persist.tile([128, IC, O], F16, tag="wq")
            wk = persist.tile([128, IC, O], F16, tag="wk")
            wv = persist.tile([128, IC, O], F16, tag="wv")
            qT = [persist.tile([128, S], F16, tag=f"qT{i}", name=f"qT{i}")
                  for i in range(2)]
            kT = [persist.tile([128, S], F16, tag=f"kT{i}", name=f"kT{i}")
                  for i in range(2)]
            vS = [persist.tile([128, NHL * (HD + 1)], F16, tag=f"v{i}",
                               name=f"v{i}") for i in range(KC)]
            # deferred probs for the first two units
            pbd = [[pbdp.tile([128, 2 * QC], F16, tag=f"pbd{u}_{kc}",
                              name=f"pbd{u}_{kc}") for kc in range(KC)]
                   for u in range(2)]

            # input DMAs: one trigger per xT row-chunk (4KB lines), one
            # per weight tensor. HBM is the constraint (~335GB/s/core), so
            # order by first use: msk (retire biases), wq+xT (pre chains),
            # then wk/wv for the filler chains.
            nc.sync.dma_start(out=msk_sb, in_=msk[:, :])
            nc.sync.dma_start(out=wq, in_=wqr)
            nc.sync.dma_start(out=wk, in_=wkr)
            for i in range(IC):
                nc.sync.dma_start(out=xt[:, i, :], in_=xTr[:, i, :])
            nc.sync.dma_start(out=wv, in_=wvr)
            nc.sync.dma_start(out=bvb_sb, in_=bvb[:, :])
            mb_sb = msk_sb[:, 0:KC]
            # ACT exp-table warmup
            dummy = consts.tile([128, 1], F32, tag="dummy")
            nc.vector.memset(dummy, 0.0)
            nc.scalar.activation(dummy, dummy, EXP)

            def bias_col(t, hp):
                return KC + hp if t == "q" else KC + 2 + hp

            # ---------- pre phase: project q0 (all) + first k0 block -----
            # i-major over 5 open [128,512] chains so the PE starts as
            # soon as xt[:,0] lands instead of waiting for the whole xT.
            # q0c0/k0c0 retire first so the first scores matmul can go.
            PRE = [("q", 0, 0), ("k", 0, 0),
                   ("q", 0, 1), ("q", 0, 2), ("q", 0, 3)]
            with tc.tile_pool(name="prep", bufs=1, space="PSUM") as prep:
                pre_ps = {}
                for (t, hp, c) in PRE:
                    pre_ps[(t, hp, c)] = prep.tile(
                        [128, 512], F32, tag=f"pre{t}{hp}_{c}",
                        name=f"pre{t}{hp}_{c}")
                for i in range(IC):
                    for (t, hp, c) in PRE:
                        w = wq if t == "q" else wk
                        nc.tensor.matmul(
                            pre_ps[(t, hp, c)],
                            lhsT=w[:, i, hp * 128:(hp + 1) * 128],
                            rhs=xt[:, i, c * 512:(c + 1) * 512],
                            start=(i == 0), stop=(i == IC - 1))
                for (t, hp, c) in PRE:
                    dest = qT[hp] if t == "q" else kT[hp]
                    bc = bias_col(t, hp)
                    nc.vector.tensor_scalar_add(
                        dest[:, c * 512:(c + 1) * 512], pre_ps[(t, hp, c)],
                        msk_sb[:, bc:bc + 1])

            # ---------- interleaved attention + filler projections -------
            with tc.tile_pool(name="scp", bufs=2, space="PSUM") as scp, \
                 tc.tile_pool(name="pvp", bufs=1, space="PSUM") as pvp, \
                 tc.tile_pool(name="wkp", bufs=1, space="PSUM") as wkp, \
                 tc.tile_pool(name="pbp", bufs=3) as pbp, \
                 tc.tile_pool(name="ovtp", bufs=3) as ovtp, \
                 tc.tile_pool(name="rcp", bufs=3) as rcp, \
                 tc.tile_pool(name="osbp", bufs=2) as osbp:

                def qk_chain(t, hp, c):
                    # 256-col half-chain: ~0.85us of PE work per filler
                    w = wq if t == "q" else wk
                    dest = qT[hp] if t == "q" else kT[hp]
                    ps = wkp.tile([128, 256], F32, tag="wk",
                                  name=f"f{t}{hp}_{c}")
                    for i in range(IC):
                        nc.tensor.matmul(
                            ps, lhsT=w[:, i, hp * 128:(hp + 1) * 128],
                            rhs=xt[:, i, c * 256:(c + 1) * 256],
                            start=(i == 0), stop=(i == IC - 1))
                    bc = bias_col(t, hp)
                    nc.vector.tensor_scalar_add(
                        dest[:, c * 256:(c + 1) * 256], ps,
                        msk_sb[:, bc:bc + 1])

                def v_chain(sc):
                    ps = wkp.tile([128, O], F32, tag="wk", name=f"fv{sc}")
                    for i in range(IC):
                        nc.tensor.matmul(
                            ps, lhsT=xt[:, i, sc * 128:(sc + 1) * 128],
                            rhs=wv[:, i, :],
                            start=(i == 0), stop=(i == IC - 1))
                    vview = vS[sc].rearrange("p (h d) -> p h d", h=NHL)
                    bvview = bvb_sb.rearrange("p (h d) -> p h d", h=NHL)
                    nc.vector.tensor_add(
                        vview[:, :, 0:HD],
                        ps.rearrange("p (h d) -> p h d", h=NHL),
                        bvview[:, :, 0:HD])
                    nc.vector.tensor_copy(
                        vview[:, :, HD:HD + 1], bvview[:, :, HD:HD + 1])

                def out_dma(osb, hp, u):
                    dst = out[u * QC:(u + 1) * QC, hp * 128:(hp + 1) * 128]
                    dst = dst.rearrange("(j p) f -> p j f", p=128)
                    nc.sync.dma_start(
                        out=dst, in_=osb.rearrange("p j e d -> p j (e d)"))

                def out_dma_half(osb, hp, u, e):
                    dst = out[u * QC:(u + 1) * QC,
                              hp * 128 + e * HD:hp * 128 + (e + 1) * HD]
                    dst = dst.rearrange("(j p) d -> p j d", p=128)
                    nc.sync.dma_start(out=dst, in_=osb[:, :, e, :])

                # emits the psum->fp16 copy now (frees the pv bank), returns
                # the transpose+normalize closure for the tail queue
                def emit_tail_half(pv_ap, osb, e, label, dma_half=None):
                    ovt = ovtp.tile([HD + 1, QC], F16, tag="ovt",
                                    name=f"ovt_{label}")
                    nc.vector.tensor_scalar_mul(ovt, pv_ap, 1.0 / 64.0)

                    def rest():
                        tr = wkp.tile([128, NJ, 128], F16, tag="wk",
                                      name=f"tr_{label}")
                        for jb in range(NJ):
                            nc.tensor.transpose(
                                tr[:, jb, 0:HD + 1],
                                ovt[:, jb * 128:(jb + 1) * 128],
                                ident[0:HD + 1, 0:HD + 1])
                        rc = rcp.tile([128, NJ], F32, tag="rc",
                                      name=f"rc_{label}")
                        nc.vector.reciprocal(rc, tr[:, :, 64])
                        for jb in range(NJ):
                            nc.vector.tensor_scalar_mul(
                                osb[:, jb, e, :], tr[:, jb, 0:HD],
                                rc[:, jb:jb + 1])
                        if dma_half is not None:
                            out_dma_half(osb, dma_half[0], dma_half[1], e)
                    return rest

                units = []   # (hp, u, mode) mode: "defer" | "live"
                for hp in range(2):
                    for u in range(NU):
                        mode = "defer" if (hp == 0 and u < 2) else "live"
                        units.append((hp, u, mode))
                NG = len(units)
                # deferred-PV host unit -> (src_unit, e)
                defmap = {2: (0, 0), 3: (0, 1), 5: (1, 0), 6: (1, 1)}
                # filler half-chains in required (just-in-time) order,
                # consumed per pops[g]
                fillers = (
                    [lambda c=c: qk_chain("k", 0, c) for c in range(2, 8)]
                    + [lambda s=s: v_chain(s) for s in range(KC)]
                    + [lambda c=c: qk_chain("q", 1, c) for c in range(2)]
                    + [lambda c=c: qk_chain("k", 1, c) for c in range(8)]
                    + [lambda c=c: qk_chain("q", 1, c) for c in range(2, 8)]
                )
                pops = {0: 12, 1: 12, 2: 4, 3: 4, 4: 4, 5: 2}
                osbs = {}
                tail_q = []
                pvs = {}

                def emit_sc(g, kc):
                    hp, u, _ = units[g]
                    sct = scp.tile([128, 2 * QC], F32, tag="sc",
                                   name=f"sc{g}_{kc}")
                    for e in range(2):
                        lo = e * 64
                        nc.tensor.matmul(
                            sct[:, e * QC:(e + 1) * QC],
                            lhsT=kT[hp][lo:lo + 64, kc * 128:(kc + 1) * 128],
                            rhs=qT[hp][lo:lo + 64, u * QC:(u + 1) * QC],
                            start=True, stop=True)
                    return sct

                steps = [(g, kc) for g in range(NG) for kc in range(KC)]
                sc_tiles = {steps[0]: emit_sc(*steps[0])}
                for si, (g, kc) in enumerate(steps):
                    hp, u, mode = units[g]
                    dv = defmap.get(g)
                    if kc == 0:
                        if mode == "live":
                            pvs[g] = [pvp.tile([HD + 1, QC], F32, tag=t,
                                               name=f"pv{t}_{g}")
                                      for t in ("pvA", "pvB")]
                            osbs[g] = osbp.tile([128, NJ, 2, HD], F32,
                                                tag="osb", name=f"osb{g}")
                        if dv is not None:
                            pvs[(g, "D")] = pvp.tile(
                                [HD + 1, QC], F32, tag="pvD", name=f"pvD_{g}")
                            if dv[0] not in osbs:
                                osbs[dv[0]] = osbp.tile(
                                    [128, NJ, 2, HD], F32, tag="osb",
                                    name=f"osb{dv[0]}")
                    sct = sc_tiles.pop((g, kc))
                    if mode == "defer":
                        pb = pbd[g][kc]
                    else:
                        pb = pbp.tile([128, 2 * QC], F16, tag="pb",
                                      name=f"pb{g}_{kc}")
                    nc.scalar.activation(
                        pb, sct, EXP, bias=mb_sb[:, kc:kc + 1], scale=0.125)
                    # one filler between the exp and the next scores tile
                    if kc < pops.get(g, 0) and fillers:
                        fillers.pop(0)()
                    # prefetch the next scores tile (ring slot frees when
                    # the exp two steps back completes)
                    if si + 1 < len(steps):
                        sc_tiles[steps[si + 1]] = emit_sc(*steps[si + 1])
                    if mode == "live":
                        for e in range(2):
                            hh = 2 * hp + e
                            nc.tensor.matmul(
                                pvs[g][e][:, :],
                                lhsT=vS[kc][:, hh * 65:hh * 65 + 65],
                                rhs=pb[:, e * QC:(e + 1) * QC],
                                start=(kc == 0), stop=(kc == KC - 1))
                    if dv is not None:
                        su, se = dv
                        shh = 2 * units[su][0] + se
                        nc.tensor.matmul(
                            pvs[(g, "D")][:, :],
                            lhsT=vS[kc][:, shh * 65:shh * 65 + 65],
                            rhs=pbd[su][kc][:, se * QC:(se + 1) * QC],
                            start=(kc == 0), stop=(kc == KC - 1))
                    if tail_q:
                        tail_q.pop(0)()
                    if kc == KC - 1:
                        # end of unit: free pv psum fast (fp16 copies),
                        # spread the rest of the tails into the next
                        # unit's stream. The last unit's tails carry
                        # their own per-half DMA for a short finish.
                        last = g == NG - 1
                        if mode == "live":
                            for e in range(2):
                                tail_q.append(emit_tail_half(
                                    pvs[g][e], osbs[g], e, f"g{g}e{e}",
                                    dma_half=(hp, u) if last else None))
                            if not last:
                                tail_q.append(
                                    lambda g=g, hp=hp, u=u: out_dma(
                                        osbs.pop(g), hp, u))
                        if dv is not None:
                            su, se = dv
                            tail_q.append(emit_tail_half(
                                pvs[(g, "D")], osbs[su], se, f"d{su}e{se}"))
                            if se == 1:
                                shp, suu, _ = units[su]
                                tail_q.append(
                                    lambda su=su, shp=shp, suu=suu: out_dma(
                                        osbs.pop(su), shp, suu))

                while tail_q:
                    tail_q.pop(0)()
    nc.finalize()
    return nc


_NC_CACHE = None


def _get_nc():
    global _NC_CACHE
    if _NC_CACHE is None:
        _NC_CACHE = build_nc()
    return _NC_CACHE


def make_in_maps(inputs, attention_mask, Wq, bq, Wk, bk, Wv, bv):
    x = np.asarray(inputs, dtype=np.float32)
    mask = np.asarray(attention_mask)
    Wq = np.asarray(Wq, dtype=np.float32)
    Wk = np.asarray(Wk, dtype=np.float32)
    Wv = np.asarray(Wv, dtype=np.float32)
    bq = np.asarray(bq, dtype=np.float32)
    bk = np.asarray(bk, dtype=np.float32)
    bv = np.asarray(bv, dtype=np.float32)

    xTb = [np.ascontiguousarray(x[b].T).astype(np.float16) for b in range(B)]
    mbb = [np.ascontiguousarray(
        ((1.0 - mask[b].astype(np.float32)) * NEG).reshape(KC, 128).T)
        for b in range(B)]
    in_maps = []
    for c in range(8):
        b, g = c // G, c % G
        cols = slice(g * O, (g + 1) * O)
        bqs, bks = bq[cols], bk[cols]
        bvc = np.concatenate(
            [np.concatenate([bv[cols][h * 64:(h + 1) * 64], [1.0]])
             for h in range(NHL)]).astype(np.float32)
        bvbc = np.ascontiguousarray(np.broadcast_to(bvc[None, :], (128, len(bvc))))
        mskc = np.concatenate(
            [mbb[b],
             np.stack([bqs[:128], bqs[128:], bks[:128], bks[128:]], axis=1)],
            axis=1)

        def pack(W):
            # [H, O] -> [128, IC*O] p-major so DMA lines are 4KB
            wt = W.T[:, cols].astype(np.float16)
            return np.ascontiguousarray(
                wt.reshape(IC, 128, O).transpose(1, 0, 2).reshape(128, IC * O))

        in_maps.append({
            "xT": xTb[b],
            "wqT": pack(Wq),
            "wkT": pack(Wk),
            "wvT": pack(Wv),
            "msk": np.ascontiguousarray(mskc),
            "bvb": bvbc.astype(np.float16),
        })
    return in_maps


def assemble(results):
    outs = [results[c]["out"] for c in range(8)]
    full = np.stack(
        [np.concatenate(outs[b * G:(b + 1) * G], axis=1) for b in range(B)])
    return np.ascontiguousarray(full.astype(np.float32))


def kernel(**inputs) -> np.ndarray:
    nc = _get_nc()
    in_maps = make_in_maps(**inputs)
    res = run_bass_kernel_spmd(nc, in_maps, core_ids=list(range(8)))
    return assemble(res.results)


# revision 17
# speedup vs baseline: 1.0261x; 1.0261x over previous
"""BertSelfAttention on 8 Trainium2 NeuronCores.

Problem: B=2, S=2048, H=1024, 16 heads x 64. Sharding: batch x head-group
(2 batches x 4 head-groups of 4 heads = 8 cores). Each core computes
q/k/v projections for its 4 heads and full attention over them.

Single interleaved instruction stream, built around two facts:
 - ACT (exp over all S^2*nhl scores) is the serial floor (~147us);
   it must never wait.
 - The PE drops from 2.4GHz to 1.2GHz whenever it idles >~100ns and
   needs ~3us of continuous work to get back to full clock, so it must
   never idle: projection chains are spread through the attention
   stream as filler work.

Structure: 8 "units" of (head-pair hp, 512 queries). Per unit, 16 key
chunks: scores[128k, 2x512q] (both heads merged; the mask bias is
per-partition = per-key, so one exp instruction covers both heads),
exp -> probs fp16, PV accumulate [65, 512] per head (65th V column = 1
-> softmax denominator). The first two units defer their PV (probs
parked in SBUF) because V isn't projected yet; four later units host
the deferred PV chains. Tails (1/64-scaled fp16 transpose via PE,
reciprocal, normalize, store) are spread into the next unit's stream.

PSUM budget (8 banks): scores ring 2x[128,1024] = 4, pv A/B/D = 3,
proj/transpose scratch = 1.
"""

import sys

sys.path.insert(0, "/opt/trn_rl_repo")

import numpy as np

import concourse.tile as tile
from concourse.masks import make_identity
from concourse import bacc, mybir
from concourse.bass_utils import run_bass_kernel_spmd

F32 = mybir.dt.float32
F16 = mybir.dt.float16
EXP = mybir.ActivationFunctionType.Exp

B, S, H = 2, 2048, 1024
NH, HD = 16, 64
G = 4                 # head-groups (cores per batch)
NHL = NH // G         # heads per core
O = NHL * HD          # 256 output features per core
IC = H // 128         # 8 contraction chunks
KC = S // 128         # 16 key chunks
QC = 512              # queries per unit
NU = S // QC          # 4 q-units per head pair
NJ = QC // 128        # 128-row blocks per unit
NEG = -1.0e30


def build_nc():
    nc = bacc.Bacc(None, target_bir_lowering=False)
    xT = nc.declare_dram_parameter("xT", [H, S], F16, isOutput=False)
    # weights host-packed p-major ([p, i*O]) so DMA lines are 4KB
    wqT = nc.declare_dram_parameter("wqT", [128, IC * O], F16, isOutput=False)
    wkT = nc.declare_dram_parameter("wkT", [128, IC * O], F16, isOutput=False)
    wvT = nc.declare_dram_parameter("wvT", [128, IC * O], F16, isOutput=False)
    # msk: cols 0:16 = additive mask bias per key chunk, 16:20 = q/k biases
    msk = nc.declare_dram_parameter("msk", [128, KC + 4], F32, isOutput=False)
    bvb = nc.declare_dram_parameter("bvb", [128, NHL * (HD + 1)], F16,
                                    isOutput=False)
    out = nc.declare_dram_parameter("out", [S, O], F32, isOutput=True)

    xTr = xT.rearrange("(i p) s -> p i s", p=128)
    wqr = wqT.rearrange("p (i o) -> p i o", i=IC)
    wkr = wkT.rearrange("p (i o) -> p i o", i=IC)
    wvr = wvT.rearrange("p (i o) -> p i o", i=IC)

    with tile.TileContext(nc) as tc:
        with tc.tile_pool(name="consts", bufs=1) as consts, \
             tc.tile_pool(name="persist", bufs=1) as persist, \
             tc.tile_pool(name="pbdp", bufs=1) as pbdp:
            ident = consts.tile([128, 128], F16, tag="ident")
            make_identity(nc, ident)
            msk_sb = consts.tile([128, KC + 4], F32, tag="msk")
            bvb_sb = consts.tile([128, NHL * (HD + 1)], F16, tag="bvb")

            xt = persist.tile([128, IC, S], F16, tag="xt")
            wq = persist.tile([128, IC, O], F16, tag="wq")
            wk = persist.tile([128, IC, O], F16, tag="wk")
            wv = persist.tile([128, IC, O], F16, tag="wv")
            qT = [persist.tile([128, S], F16, tag=f"qT{i}", name=f"qT{i}")
                  for i in range(2)]
            kT = [persist.tile([128, S], F16, tag=f"kT{i}", name=f"kT{i}")
                  for i in range(2)]
            vS = [persist.tile([128, NHL * (HD + 1)], F16, tag=f"v{i}",
                               name=f"v{i}") for i in range(KC)]
            # deferred probs for the first two units
            pbd = [[pbdp.tile([128, 2 * QC], F16, tag=f"pbd{u}_{kc}",
                              name=f"pbd{u}_{kc}") for kc in range(KC)]
                   for u in range(2)]

            # input DMAs: one trigger per xT row-chunk (4KB lines), one
            # per weight tensor. Trigger issue costs ~0.6us serially per
            # queue, so split across the two HW DGE queues (Sync + the
            # still-idle Scalar engine). HBM (~335GB/s/core) takes the
            # packets round-robin either way.
            nc.sync.dma_start(out=msk_sb, in_=msk[:, :])
            nc.sync.dma_start(out=wq, in_=wqr)
            for i in range(4):
                nc.sync.dma_start(out=xt[:, i, :], in_=xTr[:, i, :])
            for i in range(4, IC):
                nc.scalar.dma_start(out=xt[:, i, :], in_=xTr[:, i, :])
            nc.scalar.dma_start(out=wk, in_=wkr)
            nc.scalar.dma_start(out=wv, in_=wvr)
            nc.scalar.dma_start(out=bvb_sb, in_=bvb[:, :])
            mb_sb = msk_sb[:, 0:KC]
            # ACT exp-table warmup
            dummy = consts.tile([128, 1], F32, tag="dummy")
            nc.vector.memset(dummy, 0.0)
            nc.scalar.activation(dummy, dummy, EXP)

            def bias_col(t, hp):
                return KC + hp if t == "q" else KC + 2 + hp

            # ---------- pre phase: project q0 (all) + first k0 block -----
            # i-major over 5 open [128,512] chains so the PE starts as
            # soon as xt[:,0] lands instead of waiting for the whole xT.
            # q0c0/k0c0 retire first so the first scores matmul can go.
            PRE = [("q", 0, 0), ("k", 0, 0),
                   ("q", 0, 1), ("q", 0, 2), ("q", 0, 3)]
            with tc.tile_pool(name="prep", bufs=1, space="PSUM") as prep:
                pre_ps = {}
                for (t, hp, c) in PRE:
                    pre_ps[(t, hp, c)] = prep.tile(
                        [128, 512], F32, tag=f"pre{t}{hp}_{c}",
                        name=f"pre{t}{hp}_{c}")
                for i in range(IC):
                    for (t, hp, c) in PRE:
                        w = wq if t == "q" else wk
                        nc.tensor.matmul(
                            pre_ps[(t, hp, c)],
                            lhsT=w[:, i, hp * 128:(hp + 1) * 128],
                            rhs=xt[:, i, c * 512:(c + 1) * 512],
                            start=(i == 0), stop=(i == IC - 1))
                for (t, hp, c) in PRE:
                    dest = qT[hp] if t == "q" else kT[hp]
                    bc = bias_col(t, hp)
                    nc.vector.tensor_scalar_add(
                        dest[:, c * 512:(c + 1) * 512], pre_ps[(t, hp, c)],
                        msk_sb[:, bc:bc + 1])

            # ---------- interleaved attention + filler projections -------
            with tc.tile_pool(name="scp", bufs=2, space="PSUM") as scp, \
                 tc.tile_pool(name="pvp", bufs=1, space="PSUM") as pvp, \
                 tc.tile_pool(name="wkp", bufs=1, space="PSUM") as wkp, \
                 tc.tile_pool(name="pbp", bufs=3) as pbp, \
                 tc.tile_pool(name="ovtp", bufs=3) as ovtp, \
                 tc.tile_pool(name="rcp", bufs=3) as rcp, \
                 tc.tile_pool(name="osbp", bufs=2) as osbp:

                def qk_chain(t, hp, c):
                    # 256-col half-chain: ~0.85us of PE work per filler
                    w = wq if t == "q" else wk
                    dest = qT[hp] if t == "q" else kT[hp]
                    ps = wkp.tile([128, 256], F32, tag="wk",
                                  name=f"f{t}{hp}_{c}")
                    for i in range(IC):
                        nc.tensor.matmul(
                            ps, lhsT=w[:, i, hp * 128:(hp + 1) * 128],
                            rhs=xt[:, i, c * 256:(c + 1) * 256],
                            start=(i == 0), stop=(i == IC - 1))
                    bc = bias_col(t, hp)
                    nc.vector.tensor_scalar_add(
                        dest[:, c * 256:(c + 1) * 256], ps,
                        msk_sb[:, bc:bc + 1])

                def v_chain(sc):
                    ps = wkp.tile([128, O], F32, tag="wk", name=f"fv{sc}")
                    for i in range(IC):
                        nc.tensor.matmul(
                            ps, lhsT=xt[:, i, sc * 128:(sc + 1) * 128],
                            rhs=wv[:, i, :],
                            start=(i == 0), stop=(i == IC - 1))
                    vview = vS[sc].rearrange("p (h d) -> p h d", h=NHL)
                    bvview = bvb_sb.rearrange("p (h d) -> p h d", h=NHL)
                    nc.vector.tensor_add(
                        vview[:, :, 0:HD],
                        ps.rearrange("p (h d) -> p h d", h=NHL),
                        bvview[:, :, 0:HD])
                    nc.vector.tensor_copy(
                        vview[:, :, HD:HD + 1], bvview[:, :, HD:HD + 1])

                def out_dma(osb, hp, u):
                    dst = out[u * QC:(u + 1) * QC, hp * 128:(hp + 1) * 128]
                    dst = dst.rearrange("(j p) f -> p j f", p=128)
                    nc.sync.dma_start(
                        out=dst, in_=osb.rearrange("p j e d -> p j (e d)"))

                def out_dma_half(osb, hp, u, e):
                    dst = out[u * QC:(u + 1) * QC,
                              hp * 128 + e * HD:hp * 128 + (e + 1) * HD]
                    dst = dst.rearrange("(j p) d -> p j d", p=128)
                    nc.sync.dma_start(out=dst, in_=osb[:, :, e, :])

                # emits the psum->fp16 copy now (frees the pv bank), returns
                # the transpose+normalize closure for the tail queue
                def emit_tail_half(pv_ap, osb, e, label, dma_half=None,
                                   use_act=False):
                    ovt = ovtp.tile([HD + 1, QC], F16, tag="ovt",
                                    name=f"ovt_{label}")
                    nc.vector.tensor_scalar_mul(ovt, pv_ap, 1.0 / 64.0)

                    def rest():
                        tr = wkp.tile([128, NJ, 128], F16, tag="wk",
                                      name=f"tr_{label}")
                        for jb in range(NJ):
                            nc.tensor.transpose(
                                tr[:, jb, 0:HD + 1],
                                ovt[:, jb * 128:(jb + 1) * 128],
                                ident[0:HD + 1, 0:HD + 1])
                        rc = rcp.tile([128, NJ], F32, tag="rc",
                                      name=f"rc_{label}")
                        nc.vector.reciprocal(rc, tr[:, :, 64])
                        for jb in range(NJ):
                            # the very last tail runs its second half's
                            # normalize on the (by then idle) ACT engine
                            # so the two halves' muls overlap
                            if use_act:
                                nc.scalar.mul(
                                    osb[:, jb, e, :], tr[:, jb, 0:HD],
                                    rc[:, jb:jb + 1])
                            else:
                                nc.vector.tensor_scalar_mul(
                                    osb[:, jb, e, :], tr[:, jb, 0:HD],
                                    rc[:, jb:jb + 1])
                        if dma_half is not None:
                            out_dma_half(osb, dma_half[0], dma_half[1], e)
                    return rest

                units = []   # (hp, u, mode) mode: "defer" | "live"
                for hp in range(2):
                    for u in range(NU):
                        mode = "defer" if (hp == 0 and u < 2) else "live"
                        units.append((hp, u, mode))
                NG = len(units)
                # deferred-PV host unit -> (src_unit, e)
                defmap = {2: (0, 0), 3: (0, 1), 5: (1, 0), 6: (1, 1)}
                # filler half-chains in required (just-in-time) order,
                # consumed per pops[g] at evenly spread kc slots
                fillers = (
                    [lambda c=c: qk_chain("k", 0, c) for c in range(2, 8)]
                    + [lambda s=s: v_chain(s) for s in range(KC)]
                    + [lambda c=c: qk_chain("q", 1, c) for c in range(2)]
                    + [lambda c=c: qk_chain("k", 1, c) for c in range(8)]
                    + [lambda c=c: qk_chain("q", 1, c) for c in range(2, 8)]
                )
                pops = {0: 10, 1: 12, 2: 3, 3: 3, 4: 6, 5: 2, 6: 2}
                popset = {
                    g: {kc for kc in range(KC)
                        if (kc * n) // KC < ((kc + 1) * n) // KC}
                    for g, n in pops.items()}
                osbs = {}
                tail_q = []
                pvs = {}

                def emit_sc(g, kc):
                    hp, u, _ = units[g]
                    sct = scp.tile([128, 2 * QC], F32, tag="sc",
                                   name=f"sc{g}_{kc}")
                    for e in range(2):
                        lo = e * 64
                        nc.tensor.matmul(
                            sct[:, e * QC:(e + 1) * QC],
                            lhsT=kT[hp][lo:lo + 64, kc * 128:(kc + 1) * 128],
                            rhs=qT[hp][lo:lo + 64, u * QC:(u + 1) * QC],
                            start=True, stop=True)
                    return sct

                steps = [(g, kc) for g in range(NG) for kc in range(KC)]
                sc_tiles = {steps[0]: emit_sc(*steps[0])}
                for si, (g, kc) in enumerate(steps):
                    hp, u, mode = units[g]
                    dv = defmap.get(g)
                    if kc == 0:
                        if mode == "live":
                            pvs[g] = [pvp.tile([HD + 1, QC], F32, tag=t,
                                               name=f"pv{t}_{g}")
                                      for t in ("pvA", "pvB")]
                            osbs[g] = osbp.tile([128, NJ, 2, HD], F32,
                                                tag="osb", name=f"osb{g}")
                        if dv is not None:
                            pvs[(g, "D")] = pvp.tile(
                                [HD + 1, QC], F32, tag="pvD", name=f"pvD_{g}")
                            if dv[0] not in osbs:
                                osbs[dv[0]] = osbp.tile(
                                    [128, NJ, 2, HD], F32, tag="osb",
                                    name=f"osb{dv[0]}")
                    sct = sc_tiles.pop((g, kc))
                    if mode == "defer":
                        pb = pbd[g][kc]
                    else:
                        pb = pbp.tile([128, 2 * QC], F16, tag="pb",
                                      name=f"pb{g}_{kc}")
                    nc.scalar.activation(
                        pb, sct, EXP, bias=mb_sb[:, kc:kc + 1], scale=0.125)
                    # one filler between the exp and the next scores tile
                    if kc in popset.get(g, ()) and fillers:
                        fillers.pop(0)()
                    # prefetch the next scores tile (ring slot frees when
                    # the exp two steps back completes)
                    if si + 1 < len(steps):
                        sc_tiles[steps[si + 1]] = emit_sc(*steps[si + 1])
                    if mode == "live":
                        for e in range(2):
                            hh = 2 * hp + e
                            nc.tensor.matmul(
                                pvs[g][e][:, :],
                                lhsT=vS[kc][:, hh * 65:hh * 65 + 65],
                                rhs=pb[:, e * QC:(e + 1) * QC],
                                start=(kc == 0), stop=(kc == KC - 1))
                    if dv is not None:
                        su, se = dv
                        shh = 2 * units[su][0] + se
                        nc.tensor.matmul(
                            pvs[(g, "D")][:, :],
                            lhsT=vS[kc][:, shh * 65:shh * 65 + 65],
                            rhs=pbd[su][kc][:, se * QC:(se + 1) * QC],
                            start=(kc == 0), stop=(kc == KC - 1))
                    if tail_q:
                        tail_q.pop(0)()
                    if kc == KC - 1:
                        # end of unit: free pv psum fast (fp16 copies),
                        # spread the rest of the tails into the next
                        # unit's stream. The last unit's tails carry
                        # their own per-half DMA for a short finish.
                        last = g == NG - 1
                        if mode == "live":
                            for e in range(2):
                                tail_q.append(emit_tail_half(
                                    pvs[g][e], osbs[g], e, f"g{g}e{e}",
                                    dma_half=(hp, u) if last else None,
                                    use_act=last and e == 1))
                            if not last:
                                tail_q.append(
                                    lambda g=g, hp=hp, u=u: out_dma(
                                        osbs.pop(g), hp, u))
                        if dv is not None:
                            su, se = dv
                            tail_q.append(emit_tail_half(
                                pvs[(g, "D")], osbs[su], se, f"d{su}e{se}"))
                            if se == 1:
                                shp, suu, _ = units[su]
                                tail_q.append(
                                    lambda su=su, shp=shp, suu=suu: out_dma(
                                        osbs.pop(su), shp, suu))

                while tail_q:
                    tail_q.pop(0)()
    nc.finalize()
    return nc


_NC_CACHE = None


def _get_nc():
    global _NC_CACHE
    if _NC_CACHE is None:
        _NC_CACHE = build_nc()
    return _NC_CACHE


def make_in_maps(inputs, attention_mask, Wq, bq, Wk, bk, Wv, bv):
    x = np.asarray(inputs, dtype=np.float32)
    mask = np.asarray(attention_mask)
    Wq = np.asarray(Wq, dtype=np.float32)
    Wk = np.asarray(Wk, dtype=np.float32)
    Wv = np.asarray(Wv, dtype=np.float32)
    bq = np.asarray(bq, dtype=np.float32)
    bk = np.asarray(bk, dtype=np.float32)
    bv = np.asarray(bv, dtype=np.float32)

    xTb = [np.ascontiguousarray(x[b].T).astype(np.float16) for b in range(B)]
    mbb = [np.ascontiguousarray(
        ((1.0 - mask[b].astype(np.float32)) * NEG).reshape(KC, 128).T)
        for b in range(B)]
    in_maps = []
    for c in range(8):
        b, g = c // G, c % G
        cols = slice(g * O, (g + 1) * O)
        bqs, bks = bq[cols], bk[cols]
        bvc = np.concatenate(
            [np.concatenate([bv[cols][h * 64:(h + 1) * 64], [1.0]])
             for h in range(NHL)]).astype(np.float32)
        bvbc = np.ascontiguousarray(np.broadcast_to(bvc[None, :], (128, len(bvc))))
        mskc = np.concatenate(
            [mbb[b],
             np.stack([bqs[:128], bqs[128:], bks[:128], bks[128:]], axis=1)],
            axis=1)

        def pack(W):
            # [H, O] -> [128, IC*O] p-major so DMA lines are 4KB
            wt = W.T[:, cols].astype(np.float16)
            return np.ascontiguousarray(
                wt.reshape(IC, 128, O).transpose(1, 0, 2).reshape(128, IC * O))

        in_maps.append({
            "xT": xTb[b],
            "wqT": pack(Wq),
            "wkT": pack(Wk),
            "wvT": pack(Wv),
            "msk": np.ascontiguousarray(mskc),
            "bvb": bvbc.astype(np.float16),
        })
    return in_maps


def assemble(results):
    outs = [results[c]["out"] for c in range(8)]
    full = np.stack(
        [np.concatenate(outs[b * G:(b + 1) * G], axis=1) for b in range(B)])
    return np.ascontiguousarray(full.astype(np.float32))


def kernel(**inputs) -> np.ndarray:
    nc = _get_nc()
    in_maps = make_in_maps(**inputs)
    res = run_bass_kernel_spmd(nc, in_maps, core_ids=list(range(8)))
    return assemble(res.results)
